# revision 6
# baseline (speedup 1.0000x reference)
"""Conv2DMod (StyleGAN2-style modulated conv) on 8 Trainium2 NeuronCores.

Math (see reference):
    xm   = x * (1 + style)                      # per-sample, per-Cin scale
    d    = sqrt(||K_f||^2 * H*W + ||s_b||^2 + eps)   # [B,F]
    y    = conv2d_symmetric_pad(xm, K) / d[b,f]

Everything except the conv itself is a per-sample rescale along either
Cin (contraction dim) or F (output dim), and the symmetric padding is
pixel replication (channel-independent). So the whole op folds into a
plain per-sample conv with folded weights:
    W_b[ky,kx,cin,f] = K[ky,kx,cin,f] * (1 + s_b[cin]) / d[b,f]
computed on the host (0.003% of the FLOPs), leaving the device kernel a
pure dense conv: 309 GFLOP across 8 cores, batch-parallel (2 imgs/core).

Device strategy (per core):
  - x is shipped pre-transposed to channel-major [img, row, cinhalf,
    cin128, Wpad=130] (symmetric W-padding baked in; H handled by row
    clamping in the loop).
  - For each output row: accumulate 18 fp32r matmuls into a PSUM tile
    [128 pix, 256 F]: out = xt_slice[cin,pix].T @ W_tile[cin,F] over
    (cinhalf, ky, kx). fp32r = FP22 multiply / fp32 accumulate at full
    PE rate for moving dim >= 256 (~1.5e-4 rel err).
  - DVE-copy PSUM -> SBUF, DMA out in natural NHWC layout (contiguous).
"""
import numpy as np
import orjson

import concourse.bass as bass
import concourse.mybir as mybir
from concourse import tile
from concourse.bass_utils import run_bass_kernel_spmd

F32R = mybir.dt.float32r
F32 = mybir.dt.float32

B, H, W, CIN, F, KH, KW = 16, 128, 128, 256, 256, 3, 3
NCORES = 8
BL = B // NCORES  # imgs per core
WP = W + 2  # symmetric-padded width
NCH = CIN // 128  # cin partition tiles
EPS = 1e-8

# ---------------------------------------------------------------------------
# BIR wait-count legalizer: the walrus build here supports fewer sync-wait
# commands per instruction than Tile emits (self-loading fp32r Matmult: 1;
# kernel-tail Drain: one per used proc). Hoist excess waits onto NoOps
# injected just before the offender on the same engine queue (queues run
# in order, so gating is preserved).
# ---------------------------------------------------------------------------
_WAIT_LIMIT = 1


def _legalize_waits(bir: dict, limit: int = _WAIT_LIMIT) -> dict:
    ctr = 0
    for fn in bir.get("functions", []):
        for blk in fn.get("blocks", []):
            new_insts = []
            changed = False
            for ins in blk.get("instructions", []):
                si = ins.get("sync_info")
                if si:
                    waits = si.get("on_wait") or []
                    if len(waits) > limit:
                        excess, keep = waits[:-limit], waits[-limit:]
                        for i in range(0, len(excess), limit):
                            new_insts.append(
                                {
                                    "debug": ins.get("debug", 0),
                                    "engine": ins["engine"],
                                    "ins": [],
                                    "name": f"I-wfix{ctr}-{ins['name']}",
                                    "opcode": "NoOp",
                                    "outs": [],
                                    "sync_info": {
                                        "on_update": [],
                                        "on_wait": excess[i : i + limit],
                                    },
                                }
                            )
                            ctr += 1
                        si["on_wait"] = keep
                        changed = True
                new_insts.append(ins)
            if changed:
                blk["instructions"] = new_insts
    return bir


class _LegalBass(bass.Bass):
    def to_json_bytes(self):
        return orjson.dumps(_legalize_waits(orjson.loads(super().to_json_bytes())))


# ---------------------------------------------------------------------------
# Device kernel build
# ---------------------------------------------------------------------------
_NC_CACHE = {}


def _build_nc():
    if "nc" in _NC_CACHE:
        return _NC_CACHE["nc"]
    nc = _LegalBass()
    # layouts put the SBUF partition dim (cin%128) directly before the free
    # dims so every DMA is a straight linear copy
    xt = nc.dram_tensor("xt", [BL, H, 128, NCH, WP], F32R, kind="ExternalInput")
    wb = nc.dram_tensor("wb", [BL, NCH, 128, KH, KW, F], F32R, kind="ExternalInput")
    y = nc.dram_tensor("y", [BL, H, W, F], F32, kind="ExternalOutput")

    with tile.TileContext(nc) as tc:
        with (
            tc.tile_pool(name="wpool", bufs=1) as wpool,
            tc.tile_pool(name="rows", bufs=8) as rows,
            tc.tile_pool(name="outs", bufs=4) as outs,
            tc.tile_pool(name="psum", bufs=6, space="PSUM") as psum,
        ):
            # Folded per-sample weights: one tile per (img, cinhalf).
            wt = {}
            for img in range(BL):
                for ch in range(NCH):
                    t = wpool.tile([128, KH, KW, F], F32R, tag=f"w{img}{ch}")
                    nc.sync.dma_start(t[:], wb[img, ch])  # [128, kh, kw, F] exact
                    wt[img, ch] = t

            for img in range(BL):
                row_tiles = {}

                def get_row(r, img=img, row_tiles=row_tiles):
                    if r not in row_tiles:
                        t = rows.tile([128, NCH, WP], F32R)
                        nc.sync.dma_start(t[:], xt[img, r])  # [128, NCH, WP] exact
                        row_tiles[r] = t
                    return row_tiles[r]

                for yy in range(H):
                    acc = psum.tile([128, F], F32)
                    k = 0
                    for ch in range(NCH):
                        for dy in range(KH):
                            r = min(max(yy + dy - 1, 0), H - 1)
                            rt = get_row(r)
                            for dx in range(KW):
                                nc.tensor.matmul(
                                    acc[:],
                                    rt[:, ch, dx : dx + 128],
                                    wt[img, ch][:, dy, dx, :],
                                    start=(k == 0),
                                    stop=(k == KH * KW * NCH - 1),
                                )
                                k += 1
                    ot = outs.tile([128, F], F32)
                    nc.vector.tensor_copy(ot[:], acc[:])
                    nc.sync.dma_start(y[img, yy], ot[:])
    _NC_CACHE["nc"] = nc
    return nc


# ---------------------------------------------------------------------------
# Host wrapper
# ---------------------------------------------------------------------------
def _prepare(x, style, kernel):
    x = np.asarray(x, dtype=np.float32)
    style = np.asarray(style, dtype=np.float32)
    kernel = np.asarray(kernel, dtype=np.float32)

    s = style.reshape(B, CIN)
    w_sq = np.sum(np.square(kernel), axis=(0, 1, 2))  # [F]
    s_sq = np.sum(np.square(s), axis=1)  # [B]
    d = np.sqrt(w_sq[None, :] * np.float32(H * W) + s_sq[:, None] + np.float32(EPS))
    # folded per-sample weights [B, kh, kw, Cin, F]
    wbf = kernel[None] * (1.0 + s)[:, None, None, :, None] / d[:, None, None, None, :]
    wbf = np.ascontiguousarray(
        wbf.reshape(B, KH, KW, NCH, 128, F).transpose(0, 3, 4, 1, 2, 5),
        dtype=np.float32,
    )  # [B, NCH, 128, kh, kw, F]

    xp = np.pad(x, ((0, 0), (0, 0), (1, 1), (0, 0)), mode="symmetric")  # [B,H,WP,CIN]
    xt = np.ascontiguousarray(
        xp.transpose(0, 1, 3, 2).reshape(B, H, NCH, 128, WP).transpose(0, 1, 3, 2, 4),
        dtype=np.float32,
    )  # [B, H, 128, NCH, WP]
    return xt, wbf


def kernel(x, style, kernel, _trace=False, _tmpdir=None):
    xt, wbf = _prepare(x, style, kernel)
    nc = _build_nc()
    in_maps = [
        {"xt": xt[c * BL : (c + 1) * BL], "wb": wbf[c * BL : (c + 1) * BL]}
        for c in range(NCORES)
    ]
    res = run_bass_kernel_spmd(
        nc,
        in_maps,
        core_ids=list(range(NCORES)),
        trace=_trace,
        tmpdir=_tmpdir,
    )
    y = np.concatenate([res.results[c]["y"] for c in range(NCORES)], axis=0)
    LAST_RUN.clear()
    LAST_RUN.update({"exec_time_ns": res.exec_time_ns, "results": res})
    return y


LAST_RUN = {}
